# revision 41
# baseline (speedup 1.0000x reference)
"""Raw-bacc MaxPool3d kernel, v14: bf16 datapath + 3-op DVE chain.

bf16 datapath: max-pool commutes with monotone bf16 rounding, so pooling
bf16-rounded inputs yields exactly bf16(true_max) — rel err <= 2^-8, far
inside the 2e-2 gate — and halves HBM traffic, which is the binding
constraint: the 16 per-core DMA engines cap at ~26.5 GB/s each (measured
identical solo and under all-core load; 309 ns per 8 KiB packet) and run
saturated for the whole kernel.

Loads stay as one 1-MiB dma_start per parity per half-channel tile: the
DGE deals each dma_start's 32-KiB-plane runs to engines in contiguous
blocks of ceil(n_runs/16), so this shape is the only one that is both
engine-balanced and at full packet rate (0.5-MiB dmas drop to ~19 GB/s
per engine; non-multiple-of-16-run dmas land on a prefix of engines).

DVE: per tile, D-pair max (a0 vs a1, which also frees the slot for the
next load) -> H-pair -> W-pair straight into the store tile — same
element count as the v6 5-op per-parity chain but ~78 us busy vs ~89,
so DVE never gates the DMA stream. The W-pair max pays a 2x stride-2
read penalty; the deinterleaved-write alternative measured far slower.

Last two tiles: loads and DVE chains are split in half by free rows
(h%32 stripes), and the final tile gets a dedicated output buffer and a
split store, so after the last packet lands only half a chain plus half
a store remains (~6 us tail).

Fixed overheads measured: ~7.5 us startup (engine rendezvous + DGE
spin-up before the first packet) and ~7.5 us epilogue (framework resets
the full 253-semaphore file at kernel end) — both invariant to kernel
structure.
"""

import numpy as np
from ml_dtypes import bfloat16

import concourse.bass as bass
from concourse import bacc, mybir
from concourse import bass_utils

CPC = 8
D = H = W = 128
DT = mybir.dt.bfloat16
NSLOT = 4
NT = 16

_CACHE = {}


def _build_module():
    nc = bacc.Bacc("TRN2", target_bir_lowering=False, debug=False, num_devices=8)
    x = nc.dram_tensor("x", [CPC, D, H, W], DT, kind="ExternalInput").ap()
    y = nc.dram_tensor("y", [CPC, D // 2, H // 2, W // 2], DT, kind="ExternalOutput").ap()

    a0 = [nc.alloc_sbuf_tensor(f"a0_{i}", [128, 32, 128], DT).ap() for i in range(NSLOT)]
    a1 = [nc.alloc_sbuf_tensor(f"a1_{i}", [128, 32, 128], DT).ap() for i in range(NSLOT)]
    dm = nc.alloc_sbuf_tensor("dm", [128, 32, 128], DT).ap()
    hm = nc.alloc_sbuf_tensor("hm", [128, 16, 128], DT).ap()
    wm = [nc.alloc_sbuf_tensor(f"wm_{i}", [128, 16, 64], DT).ap() for i in range(2)]
    wf = nc.alloc_sbuf_tensor("wf", [128, 16, 64], DT).ap()

    a0_sems = [nc.alloc_semaphore(f"a0_sem{i}") for i in range(NSLOT)]
    a1_sems = [nc.alloc_semaphore(f"a1_sem{i}") for i in range(NSLOT)]
    wm_sems = [nc.alloc_semaphore(f"wm_sem{i}") for i in range(2)]
    wf_sem = nc.alloc_semaphore("wf_sem")
    fh_sem = nc.alloc_semaphore("fh_sem")
    fh14_sem = nc.alloc_semaphore("fh14_sem")
    rel_sem = nc.alloc_semaphore("rel_sem")
    comp_sem = nc.alloc_semaphore("comp_sem")
    compl_sem = nc.alloc_semaphore("compl_sem")

    def tile_slices(t):
        c, half = divmod(t, 2)
        return c, half * 64

    # --- SP: loads -----------------------------------------------------
    for t in range(NT):
        c, base = tile_slices(t)
        k = t % NSLOT
        if t >= NSLOT:
            nc.sync.wait_ge(rel_sem, t - NSLOT + 1)
        if t == 0:
            # split the very first dma: if the DGE generates a dma's full
            # descriptor set before doorbelling the engines, a smaller
            # first dma starts the stream earlier.
            nc.sync.dma_start(a0[0][0:64], x[0, 0:32:2]).then_inc(a0_sems[0], 16)
            nc.sync.dma_start(a0[0][64:128], x[0, 32:64:2]).then_inc(a0_sems[0], 16)
            nc.sync.dma_start(a1[0], x[0, 1:64:2]).then_inc(a1_sems[0], 16)
        elif t < NT - 2:
            nc.sync.dma_start(a0[k], x[c, base : base + 64 : 2]).then_inc(
                a0_sems[k], 16
            )
            nc.sync.dma_start(a1[k], x[c, base + 1 : base + 64 : 2]).then_inc(
                a1_sems[k], 16
            )
        else:
            # last two tiles: halve every load by free rows (h%32
            # stripes) so the DVE chain can run on the first half while
            # the second is in flight.
            hi_sem = fh14_sem if t == NT - 2 else fh_sem
            srs = [
                x[c, base + par : base + 64 : 2].rearrange(
                    "d (hb r) w -> d hb (r w)", hb=4
                )
                for par in (0, 1)
            ]
            for (buf, sems), sr in zip(((a0, a0_sems), (a1, a1_sems)), srs):
                nc.sync.dma_start(buf[k][:, 0:16, :], sr[:, :, 0:2048]).then_inc(
                    sems[k], 16
                )
            for (buf, _), sr in zip(((a0, a0_sems), (a1, a1_sems)), srs):
                nc.sync.dma_start(buf[k][:, 16:32, :], sr[:, :, 2048:4096]).then_inc(
                    hi_sem, 16
                )

    # --- DVE: D-pair, H-pair, W-pair max -------------------------------
    # (A W-deinterleaving hmax write was tried to make wmax stride-1;
    # strided DVE writes are far slower than the stride-2 read penalty.)
    def hmax(hv_rows, src):
        hv = hm[:, hv_rows[0] : hv_rows[1], :]
        nc.vector.tensor_max(hv, src[:, 0::2, :], src[:, 1::2, :])
        return hv

    def wmax(dst, hv):
        # stride-2 reads cost 2x but every alternative measured worse:
        # deinterleaved DVE writes are far slower, pool_max is 4-byte
        # only, and tensor_reduce(axis=X) is 2x slower still (single
        # read stream, 2282 ns vs 1216 for the same 1024 outputs).
        wp = hv.rearrange("p r (w2 two) -> p r w2 two", two=2)
        return nc.vector.tensor_max(dst, wp[:, :, :, 0], wp[:, :, :, 1])

    def chain(dst, rows, n, av0, av1):
        dmv = dm[:, rows : rows + n, :]
        nc.vector.tensor_max(dmv, av0[:, rows : rows + n, :], av1[:, rows : rows + n, :])
        hv = hmax((rows // 2, rows // 2 + n // 2), dmv)
        return wmax(dst, hv)

    wm_uses = [0, 0]
    for t in range(NT):
        k = t % NSLOT
        m = t % 2
        uses = t // NSLOT + 1
        if t < NT - 2:
            # slot 0's first use took two a0 dmas (32 incs, not 16)
            nc.vector.wait_ge(a0_sems[k], 16 * uses + (16 if k == 0 else 0))
            nc.vector.wait_ge(a1_sems[k], 16 * uses)
            nc.vector.tensor_max(dm, a0[k], a1[k]).then_inc(rel_sem, 1)
            hv = hmax((0, 16), dm)
            if wm_uses[m] > 0:
                nc.vector.wait_ge(wm_sems[m], 16 * wm_uses[m])
            wmax(wm[m], hv).then_inc(comp_sem, 1)
            wm_uses[m] += 1
        else:
            # last two tiles: low halves landed (16 incs on slot sems)
            hi_sem = fh14_sem if t == NT - 2 else fh_sem
            dst = wm[m] if t == NT - 2 else wf
            nc.vector.wait_ge(a0_sems[k], 16 * (uses - 1) + 16)
            nc.vector.wait_ge(a1_sems[k], 16 * (uses - 1) + 16)
            if t == NT - 2 and wm_uses[m] > 0:
                nc.vector.wait_ge(wm_sems[m], 16 * wm_uses[m])
            lo = chain(dst[:, 0:8, :], 0, 16, a0[k], a1[k])
            if t == NT - 1:
                lo.then_inc(compl_sem, 1)
            nc.vector.wait_ge(hi_sem, 32)
            chain(dst[:, 8:16, :], 16, 16, a0[k], a1[k]).then_inc(comp_sem, 1)
            if t == NT - 2:
                wm_uses[m] += 1

    # --- ACT: stores ---------------------------------------------------
    for t in range(NT):
        c, base = tile_slices(t)
        m = t % 2
        if t < NT - 1:
            nc.scalar.wait_ge(comp_sem, t + 1)
            nc.scalar.dma_start(y[c, base // 2 : base // 2 + 32], wm[m]).then_inc(
                wm_sems[m], 16
            )
        else:
            # split final store: low free-rows half as soon as ready.
            yv = y[c, base // 2 : base // 2 + 32].rearrange(
                "dd (q j) ww -> dd q (j ww)", q=4
            )
            nc.scalar.wait_ge(compl_sem, 1)
            nc.scalar.dma_start(yv[:, :, 0:512], wf[:, 0:8, :]).then_inc(wf_sem, 16)
            nc.scalar.wait_ge(comp_sem, t + 1)
            nc.scalar.dma_start(yv[:, :, 512:1024], wf[:, 8:16, :]).then_inc(wf_sem, 16)
    nc.scalar.wait_ge(wm_sems[0], 16 * (NT // 2))
    nc.scalar.wait_ge(wm_sems[1], 16 * (NT // 2 - 1))
    nc.scalar.wait_ge(wf_sem, 32)

    nc.compile()
    return nc


def _get_module():
    if "nc" not in _CACHE:
        _CACHE["nc"] = _build_module()
    return _CACHE["nc"]


def _shard_inputs(x: np.ndarray) -> list[dict]:
    B, C, d, h, w = x.shape
    assert (B, C, d, h, w) == (2, 32, 128, 128, 128), x.shape
    xb = np.ascontiguousarray(x, dtype=np.float32).reshape(B * C, d, h, w)
    xb = xb.astype(bfloat16)
    return [{"x": np.ascontiguousarray(xb[i * CPC : (i + 1) * CPC])} for i in range(8)]


def _gather_output(results) -> np.ndarray:
    out = np.concatenate([r["y"] for r in results], axis=0)
    return out.astype(np.float32).reshape(2, 32, D // 2, H // 2, W // 2)


def kernel(x: np.ndarray) -> np.ndarray:
    nc = _get_module()
    in_maps = _shard_inputs(x)
    res = bass_utils.run_bass_kernel_spmd(nc, in_maps, core_ids=list(range(8)))
    return _gather_output(res.results)
